# revision 19
# baseline (speedup 1.0000x reference)
"""Trainium2 Bass kernel for nn_MultiHeadCrossAttention (BS=4, S=512, DM=512, H=8).

Sharding: one attention head per NeuronCore (8 heads / 8 cores). Each core
receives the full (transposed) q/k/v plus its head's weight slices, computes
its head end-to-end including the rank-64 slice of the output projection, and
the host sums the 8 partial outputs.

v5 structure:
  - Weights packed into 3 combined DMAs (each dma_start costs ~0.7us of
    queue-issue occupancy) on the scalar HWDGE queue; inputs lead sync.
  - Single PSUM rotation [128,4,512] x 2 bufs; P2 software-pipelined
    (E mms of unit n+1 precede the fenmu/wrec/rt tail of unit n) so the
    ~2us exp activation streams back-to-back; vh / q-proj piggyback on
    consumed P2 tiles (slot 1 / slot 3) to preserve rotation parity.
  - All P3 work post-loop, pipelined across b (acts never interrupt the
    P2 exp stream); ONE merged stats chain; P5 elementwise split DVE/GPSIMD;
    P6 packed K=128 with single [128,4,512] ACT copies.
"""

import numpy as np

BS, S, DM, H, DK = 4, 512, 512, 8, 64
EPS = 1e-6
NCORES = 8


def build_program(nc, tile, mybir, bo_zero):
    f32 = mybir.dt.float32
    bf16 = mybir.dt.bfloat16
    i32 = mybir.dt.int32
    AF = mybir.ActivationFunctionType
    OP = mybir.AluOpType
    AX = mybir.AxisListType

    # ---- DRAM I/O ----
    qT_d = nc.dram_tensor("qT", [BS, 128, 4, S], bf16, kind="ExternalInput")
    kT_d = nc.dram_tensor("kT", [BS, 128, 4, S], bf16, kind="ExternalInput")
    vT_d = nc.dram_tensor("vT", [BS, 128, 4, S], bf16, kind="ExternalInput")
    # packed weights: [Wq|Wk|Wv (4x64 each) | WoP (4x128) | id (128)] bf16
    wb_d = nc.dram_tensor("wb", [128, 1408], bf16, kind="ExternalInput")
    # packed f32: [idf (128) | bo2 (4)]
    wf_d = nc.dram_tensor("wf", [128, 132], f32, kind="ExternalInput")
    # packed small f32 (64 partitions): [bqc | bkc | alpha | beta4]
    sb_d = nc.dram_tensor("sb", [DK, 4], f32, kind="ExternalInput")
    bv_d = nc.dram_tensor("bv", [1, DK], bf16, kind="ExternalInput")
    outT_d = nc.dram_tensor("outT", [BS, DM, S], bf16, kind="ExternalOutput")

    with tile.TileContext(nc) as tc:
        with (
            tc.tile_pool(name="persist", bufs=1) as pp,
            tc.tile_pool(name="consts", bufs=1) as cp,
            tc.tile_pool(name="kin", bufs=1) as kip,
            tc.tile_pool(name="vin", bufs=1) as vip,
            tc.tile_pool(name="qin", bufs=2) as qip,
            tc.tile_pool(name="exw", bufs=3) as exp_pool,
            tc.tile_pool(name="wrw", bufs=3) as wrp,
            tc.tile_pool(name="p5w", bufs=2) as p5p,
            tc.tile_pool(name="otw", bufs=2) as otp,
            tc.tile_pool(name="stats", bufs=1) as stp,
            tc.tile_pool(name="psum", bufs=1, space="PSUM") as psp,
        ):
            # ---- persistent SBUF ----
            qhT = pp.tile([DK, BS, S], bf16, tag="qhT")
            khT = pp.tile([DK, BS, S], bf16, tag="khT")
            hq = pp.tile([128, BS, S], bf16, tag="hq")
            vh_all = pp.tile([128, 4, BS, DK], bf16, tag="vh")   # [j, jc, c, d]
            rt_all = pp.tile([128, BS, 4, S], bf16, tag="rt")    # [j, b, jc, i]
            e_all = pp.tile([128, BS, 2, 1024], bf16, tag="e")
            Z_all = pp.tile([128, 64], f32, tag="Z")   # cols = b*16 + ic*4 + c
            Q_all = pp.tile([128, 64], f32, tag="Q")
            w1_all = pp.tile([128, 64], f32, tag="w1")
            w0_all = pp.tile([128, 16], f32, tag="w0")  # cols = b*4 + ic
            w0T = pp.tile([16, 128], f32, tag="w0T")
            w0f = pp.tile([1, 16 * 128], f32, tag="w0f")
            bvb = pp.tile([128, DK], bf16, tag="bvb")

            wb = cp.tile([128, 1408], bf16, tag="wb")
            wf = cp.tile([128, 132], f32, tag="wf")
            sb = cp.tile([DK, 4], f32, tag="sb")
            bv_s = cp.tile([1, DK], bf16, tag="bv")
            ones_b = cp.tile([1, 128], bf16, tag="ones_b")
            ones_f = cp.tile([1, 128], f32, tag="ones_f")
            warm_z = cp.tile([128, S], bf16, tag="warm_z")

            def Wq_c(mc):
                return wb[:, mc * DK:(mc + 1) * DK]

            def Wk_c(mc):
                return wb[:, 256 + mc * DK:256 + (mc + 1) * DK]

            def Wv_c(mc):
                return wb[:, 512 + mc * DK:512 + (mc + 1) * DK]

            def WoP_c(nch):
                return wb[:, 768 + nch * 128:768 + (nch + 1) * 128]

            id_s = wb[:, 1280:1408]
            idf_s = wf[:, 0:128]
            bqc_s, bkc_s = sb[:, 0:1], sb[:, 1:2]
            al_s, b4_s = sb[:, 2:3], sb[:, 3:4]

            # ---- DMA: big inputs lead the sync queue ----
            ktiles, qtiles, vtiles = [], [None] * BS, [None] * BS
            for b in range(BS):
                kt = kip.tile([128, 4, S], bf16, tag=f"kt{b}")
                nc.sync.dma_start(kt[:], kT_d[b])
                ktiles.append(kt)
            for b in range(BS):
                qt = qip.tile([128, 4, S], bf16, tag="qt", name=f"qt{b}")
                nc.sync.dma_start(qt[:], qT_d[b])
                qtiles[b] = qt
                vt = vip.tile([128, 4, S], bf16, tag=f"vt{b}")
                nc.sync.dma_start(vt[:], vT_d[b])
                vtiles[b] = vt
            # packed weights on the scalar (ACT) HWDGE queue in parallel
            nc.scalar.dma_start(wb[:], wb_d[:])
            nc.scalar.dma_start(wf[:], wf_d[:])
            nc.scalar.dma_start(sb[:], sb_d[:])
            nc.scalar.dma_start(bv_s[:], bv_d[:])

            nc.vector.memset(ones_b[:], 1.0)
            nc.vector.memset(ones_f[:], 1.0)
            nc.vector.memset(warm_z[:], 0.0)

            # ---- single PSUM rotation: [128, 4, 512] x 2 bufs = 8 banks ----
            def psum_t(name="ppe"):
                return psp.tile([128, 4, S], f32, tag="pe", bufs=2, name=name)

            # ---- PE warmup (HAM clock-gate) during kT0 transfer ----
            wps = psum_t("warm")
            for w in range(8):
                nc.tensor.matmul(wps[:, w % 4, :], id_s, warm_z[:],
                                 start=True, stop=True)

            # ---- P1 projections (slot 0 own tile, or slot 3 borrowed) ----
            def emit_proj(W_c, b_c, srct, dst_ap, on_act, ph=None):
                slot = 3 if ph is not None else 0
                ps = ph if ph is not None else psum_t("pproj")
                for mc in range(4):
                    nc.tensor.matmul(ps[0:DK, slot, :], W_c(mc),
                                     srct[:, mc, :],
                                     start=(mc == 0), stop=(mc == 3))
                if on_act:
                    nc.scalar.activation(dst_ap, ps[0:DK, slot, :],
                                         AF.Identity, bias=b_c)
                else:
                    nc.vector.tensor_scalar(dst_ap, ps[0:DK, slot, :], b_c,
                                            None, op0=OP.add)

            def emit_qproj(b, ph=None):
                emit_proj(Wq_c, bqc_s, qtiles[b], qhT[:, b, :], b == 0, ph)
                (nc.scalar if b == 0 else nc.sync).dma_start(
                    hq[64:128, b, :], qhT[:, b, :])

            # ---- bvb: broadcast bv across partitions via K=1 matmul ----
            def emit_bvb():
                pb = psum_t("pbv")
                nc.tensor.matmul(pb[:, 0, 0:DK], ones_b[:, :], bv_s[:],
                                 start=True, stop=True)
                nc.vector.tensor_copy(bvb[:], pb[:, 0, 0:DK])

            # ---- vh per c (borrows slot 1 of a consumed P2 tile) ----
            def emit_vh(c, ph):
                vt = vtiles[c]
                for jc in range(4):
                    for mc in range(4):
                        nc.tensor.matmul(
                            ph[:, 1, jc * DK:(jc + 1) * DK],
                            vt[:, mc, jc * 128:(jc + 1) * 128],
                            Wv_c(mc),
                            start=(mc == 0), stop=(mc == 3),
                        )
                nc.vector.tensor_tensor(
                    vh_all[:, :, c, :],
                    ph[:, 1, 0:4 * DK].rearrange("p (jc d) -> p jc d", d=DK),
                    bvb[:].unsqueeze(1).broadcast_to((128, 4, DK)),
                    op=OP.add)

            # ---- P2 unit (b, jc), software-pipelined halves ----
            def emit_e_part(b, jc):
                ph = psum_t("ppe")
                js = slice(jc * 128, (jc + 1) * 128)
                for kb in range(4):
                    nc.tensor.matmul(ph[:, kb, :], khT[:, kb, js],
                                     qhT[:, b, :], start=True, stop=True)
                ex = exp_pool.tile([128, 4, S], bf16, tag="ex")
                nc.scalar.activation(ex[:], ph[:], AF.Exp)
                return ph, ex

            def emit_fen_part(b, jc, ph, ex):
                for kb in range(4):
                    nc.tensor.matmul(ph[:, 0, :], id_s, ex[:, kb, :],
                                     start=(kb == 0), stop=(kb == 3))
                wr = wrp.tile([128, S], f32, tag="wr")
                nc.vector.reciprocal_approx_fast(wr[:], ph[:, 0, :])
                nc.gpsimd.tensor_tensor(rt_all[:, b, jc, :], ex[:, b, :],
                                        wr[:], op=OP.mult)

            # ---- P3 per b: scores + exp (own tile, post-loop) ----
            def emit_p3_mms_act(b):
                ps3 = psum_t("p3")
                for ic in range(4):
                    for jc in range(4):
                        nc.tensor.matmul(
                            ps3[:, ic // 2, (ic % 2) * 256:(ic % 2 + 1) * 256],
                            rt_all[:, b, jc, ic * 128:(ic + 1) * 128],
                            vh_all[:, jc].rearrange("p c d -> p (c d)"),
                            start=(jc == 0), stop=(jc == 3),
                        )
                nc.scalar.activation(
                    e_all[:, b, 0, :].rearrange("p (a i) -> p a i", a=2),
                    ps3[:, 0:2, :], AF.Exp)

            def emit_p3_dve(b):
                nc.gpsimd.tensor_tensor(e_all[:, b, 1, :], e_all[:, b, 0, :],
                                        e_all[:, b, 0, :], op=OP.mult)
                nc.vector.tensor_reduce(
                    Z_all[:, b * 16:(b + 1) * 16],
                    e_all[:, b, 0, :].rearrange("p (g d) -> p g d", d=DK),
                    axis=AX.X, op=OP.add)
                nc.vector.tensor_reduce(
                    Q_all[:, b * 16:(b + 1) * 16],
                    e_all[:, b, 1, :].rearrange("p (g d) -> p g d", d=DK),
                    axis=AX.X, op=OP.add)

            # ---- P4: ONE merged stats chain over all 64 cols ----
            def emit_stats():
                Zs, Qs = Z_all[:, :], Q_all[:, :]
                t = stp.tile([128, 64], f32, tag="t", name="t")
                nc.vector.tensor_tensor(t[:], Zs, Zs, op=OP.mult)
                s = stp.tile([128, 64], f32, tag="s", name="s")
                nc.vector.scalar_tensor_tensor(
                    s[:], t[:], -1.0 / DK, Qs, op0=OP.mult, op1=OP.add)
                rinv = stp.tile([128, 64], f32, tag="rinv", name="rinv")
                nc.vector.reciprocal(rinv[:], t[:])
                v63 = stp.tile([128, 64], f32, tag="v63", name="v63")
                nc.vector.tensor_tensor(v63[:], s[:], rinv[:], op=OP.mult)
                r_ = stp.tile([128, 64], f32, tag="r_", name="r_")
                nc.vector.tensor_scalar(r_[:].bitcast(i32), v63[:].bitcast(i32),
                                        1, None, op0=OP.logical_shift_right)
                nc.vector.tensor_scalar(r_[:].bitcast(i32), r_[:].bitcast(i32),
                                        -1, 0x5F3759DF, op0=OP.mult, op1=OP.add)
                nt = stp.tile([128, 64], f32, tag="nt", name="nt")
                for _ in range(2):
                    nc.vector.tensor_tensor(nt[:], v63[:], r_[:], op=OP.mult)
                    nc.vector.tensor_tensor(nt[:], nt[:], r_[:], op=OP.mult)
                    nc.vector.tensor_scalar(nt[:], nt[:], -0.5, 1.5,
                                            op0=OP.mult, op1=OP.add)
                    nc.vector.tensor_tensor(r_[:], r_[:], nt[:], op=OP.mult)
                R_ = stp.tile([128, 64], f32, tag="R_", name="R_")
                nc.vector.tensor_scalar(R_[:], r_[:], float(np.sqrt(DK - 1.0)),
                                        None, op0=OP.mult)
                u_ = stp.tile([128, 64], f32, tag="u_", name="u_")
                nc.vector.tensor_scalar(u_[:], R_[:], -EPS, 1.0,
                                        op0=OP.mult, op1=OP.add)
                g = stp.tile([128, 64], f32, tag="g", name="g")
                nc.vector.tensor_tensor(g[:], R_[:], u_[:], op=OP.mult)
                zr = stp.tile([128, 64], f32, tag="zr", name="zr")
                nc.vector.reciprocal(zr[:], Zs)
                nc.vector.tensor_tensor(w1_all[:, :], g[:], zr[:], op=OP.mult)
                gs = stp.tile([128, 16], f32, tag="gs", name="gs")
                nc.vector.tensor_reduce(
                    gs[:], g[:].rearrange("p (s c) -> p s c", c=4), axis=AX.X,
                    op=OP.add)
                nc.vector.tensor_scalar(w0_all[:, :], gs[:],
                                        -1.0 / DK, None, op0=OP.mult)
                pw = psum_t("pw")
                nc.tensor.matmul(pw[:16, 0, 0:128], w0_all[:, :],
                                 idf_s, is_transpose=True, start=True,
                                 stop=True)
                nc.vector.tensor_copy(w0T[:, :], pw[:16, 0, 0:128])
                nc.sync.dma_start(
                    w0f[0:1, :].rearrange("o (s f) -> o s f", s=16),
                    w0T[:, :])

            # ---- P5 per b ----
            def emit_p5_prep(b):
                w1e = p5p.tile([128, 16, DK], bf16, tag="w1e")
                nc.vector.tensor_copy(
                    w1e[:],
                    w1_all[:, b * 16:(b + 1) * 16].unsqueeze(-1)
                    .broadcast_to((128, 16, DK)))
                bsc = p5p.tile([128, 4, 4, DK], bf16, tag="bsc")
                nc.vector.tensor_tensor(
                    bsc[:].rearrange("p i c d -> p (i c d)"),
                    e_all[:, b, 0, :],
                    w1e[:].rearrange("p (i c) d -> p (i c d)", c=4),
                    op=OP.mult)
                t01 = p5p.tile([128, 4, DK], bf16, tag="t01")
                nc.gpsimd.tensor_tensor(t01[:], bsc[:, :, 0, :],
                                        bsc[:, :, 1, :], op=OP.add)
                t23 = p5p.tile([128, 4, DK], bf16, tag="t23")
                nc.gpsimd.tensor_tensor(t23[:], bsc[:, :, 2, :],
                                        bsc[:, :, 3, :], op=OP.add)
                ball = p5p.tile([128, 4, DK], bf16, tag="ball")
                nc.gpsimd.tensor_tensor(ball[:], t01[:], t23[:], op=OP.add)
                return ball

            def emit_p5(b, ball):
                pbig = psum_t("p5")
                for ic in range(4):
                    nc.tensor.matmul(pbig[0:64, 0, ic * 128:(ic + 1) * 128],
                                     ball[:, ic, :], id_s,
                                     start=True, stop=False,
                                     skip_group_check=True)
                    slot = b * 4 + ic
                    nc.tensor.matmul(
                        pbig[0:64, 0, ic * 128:(ic + 1) * 128],
                        ones_f[:, 0:DK],
                        w0f[0:1, slot * 128:(slot + 1) * 128],
                        start=False, stop=True, skip_group_check=True,
                    )
                nc.vector.tensor_scalar(
                    hq[0:64, b, :], pbig[0:64, 0, :],
                    al_s, b4_s, op0=OP.mult, op1=OP.add,
                )

            # ---- P6 per b: out = WoP^T @ [heads; qhT] (+bo), copies on ACT
            def emit_p6(b):
                po = psum_t("p6")
                for nch in range(4):
                    nc.tensor.matmul(po[:, nch, :], WoP_c(nch),
                                     hq[:, b, :], start=True, stop=True)
                ot = otp.tile([128, 4, S], bf16, tag="ot")
                if bo_zero:
                    nc.scalar.activation(ot[:], po[:], AF.Identity)
                else:
                    for nch in range(4):
                        nc.scalar.activation(ot[:, nch, :], po[:, nch, :],
                                             AF.Identity,
                                             bias=wf[:, 128 + nch:129 + nch])
                nc.sync.dma_start(
                    outT_d[b].rearrange("(n p) i -> p n i", p=128), ot[:])

            # ---- emission schedule ----
            for kb in range(4):
                emit_proj(Wk_c, bkc_s, ktiles[kb], khT[:, kb, :], True)
                if kb == 1:
                    # Exp table load in the ACT gap between k-proj copies
                    nc.scalar.activation(warm_z[0:1, 0:8], warm_z[0:1, 0:8],
                                         AF.Exp)
            emit_qproj(0)
            emit_bvb()

            # extras at loop index i borrow the (just consumed) PSUM tile of
            # unit i-1: vh uses slot 1, q-proj uses slot 3 — preserving the
            # 2-buffer rotation parity of the E-unit stream.
            extras = {
                2: [lambda ph: emit_vh(0, ph)],
                3: [lambda ph: emit_qproj(1, ph)],
                4: [lambda ph: emit_vh(1, ph)],
                5: [lambda ph: emit_vh(2, ph)],
                6: [lambda ph: emit_vh(3, ph)],
                7: [lambda ph: emit_qproj(2, ph)],
                10: [lambda ph: emit_qproj(3, ph)],
            }
            units = [(b, jc) for b in range(BS) for jc in range(4)]
            pend = None
            for i, (b, jc) in enumerate(units):
                ph, ex = emit_e_part(b, jc)
                if pend is not None:
                    emit_fen_part(*pend)
                    for fn in extras.get(i, []):
                        fn(pend[2])
                pend = (b, jc, ph, ex)
            emit_fen_part(*pend)

            # ---- tail: P3 pipeline, merged stats, P5/P6 pipeline ----
            emit_p3_mms_act(0)
            emit_p3_mms_act(1)
            emit_p3_dve(0)
            emit_p3_mms_act(2)
            emit_p3_dve(1)
            emit_p3_mms_act(3)
            emit_p3_dve(2)
            emit_p3_dve(3)
            emit_stats()
            for b in range(BS):
                ball = emit_p5_prep(b)
                emit_p5(b, ball)
                emit_p6(b)

    return nc


def _build(bo_zero):
    import concourse.bass as bass  # noqa
    import concourse.tile as tile
    from concourse import bacc, mybir

    nc = bacc.Bacc("TRN2", target_bir_lowering=False, debug=False,
                   num_devices=NCORES)
    build_program(nc, tile, mybir, bo_zero)
    nc.compile()
    return nc


_cached_nc = None
_cached_bo_zero = None


def make_in_maps(q, k, v, Wq, bq, Wk, bk, Wv, bv, Wo, bo, alpha, beta):
    import ml_dtypes
    bft = ml_dtypes.bfloat16

    def prelay(x):
        # [S, DM] per batch -> transposed [DM, S] -> [128, 4, S] layout
        xT = np.swapaxes(np.asarray(x, np.float32), 1, 2)  # [B, DM, S]
        return np.ascontiguousarray(
            xT.reshape(BS, 4, 128, S).transpose(0, 2, 1, 3)).astype(bft)

    def wlay(W):  # [DM, DK] -> [128, (4, DK)]
        return np.ascontiguousarray(
            np.asarray(W, np.float32).reshape(4, 128, DK).transpose(1, 0, 2)
        ).reshape(128, 4 * DK)

    qT, kT, vT = prelay(q), prelay(k), prelay(v)
    Wq, Wk, Wv, Wo = (np.asarray(x, np.float32) for x in (Wq, Wk, Wv, Wo))
    bq, bk, bv, bo = (np.asarray(x, np.float32) for x in (bq, bk, bv, bo))
    alpha, beta = np.asarray(alpha, np.float32), np.asarray(beta, np.float32)
    scale = np.float32(1.0 / np.sqrt(np.float32(DK)))  # fenmu sqrt(DK) -> Wv
    in_maps = []
    for h in range(NCORES):
        sl = slice(h * DK, (h + 1) * DK)
        wbp = np.zeros((128, 1408), np.float32)
        wbp[:, 0:256] = wlay(Wq[:, sl])
        wbp[:, 256:512] = wlay(Wk[:, sl])
        wbp[:, 512:768] = wlay(Wv[:, sl] * scale)
        for nch in range(4):
            wbp[0:64, 768 + nch * 128:768 + (nch + 1) * 128] = \
                Wo[sl, nch * 128:(nch + 1) * 128]
            wbp[64:128, 768 + nch * 128:768 + (nch + 1) * 128] = \
                4.0 * Wo[sl, nch * 128:(nch + 1) * 128]
        wbp[:, 1280:1408] = np.eye(128, dtype=np.float32)
        wfp = np.zeros((128, 132), np.float32)
        wfp[:, 0:128] = np.eye(128, dtype=np.float32)
        wfp[:, 128:132] = (bo if h == 0 else np.zeros_like(bo)).reshape(
            4, 128).T
        sbp = np.stack([bq[sl], bk[sl], alpha, 4.0 * beta], axis=1)
        in_maps.append({
            "qT": qT, "kT": kT, "vT": vT,
            "wb": wbp.astype(bft),
            "wf": wfp.astype(np.float32),
            "sb": np.ascontiguousarray(sbp).astype(np.float32),
            "bv": np.ascontiguousarray(bv[sl] * scale)[None, :].astype(bft),
        })
    return in_maps


def assemble(results):
    out = np.zeros((BS, S, DM), np.float32)
    for r in results:
        out += np.swapaxes(np.asarray(r["outT"], np.float32), 1, 2)
    return out


def kernel(**inputs) -> np.ndarray:
    global _cached_nc, _cached_bo_zero
    from concourse.bass_utils import run_bass_kernel_spmd

    bo_zero = bool(np.all(np.asarray(inputs["bo"]) == 0.0))
    if _cached_nc is None or _cached_bo_zero != bo_zero:
        _cached_nc = _build(bo_zero)
        _cached_bo_zero = bo_zero
    in_maps = make_in_maps(**inputs)
    res = run_bass_kernel_spmd(_cached_nc, in_maps, list(range(NCORES)))
    return assemble(res.results)


# revision 20
# speedup vs baseline: 1.0975x; 1.0975x over previous
"""Trainium2 Bass kernel for nn_MultiHeadCrossAttention (BS=4, S=512, DM=512, H=8).

Sharding: one attention head per NeuronCore (8 heads / 8 cores). Each core
receives the full (transposed) q/k/v plus its head's weight slices, computes
its head end-to-end including the rank-64 slice of the output projection, and
the host sums the 8 partial outputs.

v5 structure:
  - Weights packed into 3 combined DMAs (each dma_start costs ~0.7us of
    queue-issue occupancy) on the scalar HWDGE queue; inputs lead sync.
  - Single PSUM rotation [128,4,512] x 2 bufs; P2 software-pipelined
    (E mms of unit n+1 precede the fenmu/wrec/rt tail of unit n) so the
    ~2us exp activation streams back-to-back; vh / q-proj piggyback on
    consumed P2 tiles (slot 1 / slot 3) to preserve rotation parity.
  - All P3 work post-loop, pipelined across b (acts never interrupt the
    P2 exp stream); ONE merged stats chain; P5 elementwise split DVE/GPSIMD;
    P6 packed K=128 with single [128,4,512] ACT copies.
"""

import numpy as np

BS, S, DM, H, DK = 4, 512, 512, 8, 64
EPS = 1e-6
NCORES = 8


def build_program(nc, tile, mybir, bo_zero):
    f32 = mybir.dt.float32
    bf16 = mybir.dt.bfloat16
    i32 = mybir.dt.int32
    AF = mybir.ActivationFunctionType
    OP = mybir.AluOpType
    AX = mybir.AxisListType

    # ---- DRAM I/O ----
    qT_d = nc.dram_tensor("qT", [BS, 128, 4, S], bf16, kind="ExternalInput")
    kT_d = nc.dram_tensor("kT", [BS, 128, 4, S], bf16, kind="ExternalInput")
    vT_d = nc.dram_tensor("vT", [BS, 128, 4, S], bf16, kind="ExternalInput")
    # packed weights: [Wq|Wk|Wv (4x64 each) | WoP (4x128) | id (128)] bf16
    wb_d = nc.dram_tensor("wb", [128, 1408], bf16, kind="ExternalInput")
    # packed f32: [idf (128) | bo2 (4)]
    wf_d = nc.dram_tensor("wf", [128, 132], f32, kind="ExternalInput")
    # packed small f32 (64 partitions): [bqc | bkc | alpha | beta4]
    sb_d = nc.dram_tensor("sb", [DK, 4], f32, kind="ExternalInput")
    bv_d = nc.dram_tensor("bv", [1, DK], bf16, kind="ExternalInput")
    outT_d = nc.dram_tensor("outT", [BS, DM, S], bf16, kind="ExternalOutput")

    with tile.TileContext(nc) as tc:
        with (
            tc.tile_pool(name="persist", bufs=1) as pp,
            tc.tile_pool(name="consts", bufs=1) as cp,
            tc.tile_pool(name="kin", bufs=1) as kip,
            tc.tile_pool(name="vin", bufs=1) as vip,
            tc.tile_pool(name="qin", bufs=2) as qip,
            tc.tile_pool(name="exw", bufs=3) as exp_pool,
            tc.tile_pool(name="wrw", bufs=3) as wrp,
            tc.tile_pool(name="p5w", bufs=2) as p5p,
            tc.tile_pool(name="otw", bufs=2) as otp,
            tc.tile_pool(name="stats", bufs=1) as stp,
            tc.tile_pool(name="psum", bufs=1, space="PSUM") as psp,
        ):
            # ---- persistent SBUF ----
            qhT = pp.tile([DK, BS, S], bf16, tag="qhT")
            khT = pp.tile([DK, BS, S], bf16, tag="khT")
            hq = pp.tile([128, BS, S], bf16, tag="hq")
            vh_all = pp.tile([128, 4, BS, DK], bf16, tag="vh")   # [j, jc, c, d]
            rt_all = pp.tile([128, BS, 4, S], bf16, tag="rt")    # [j, b, jc, i]
            e_all = pp.tile([128, BS, 2, 1024], bf16, tag="e")
            Z_all = pp.tile([128, 64], f32, tag="Z")   # cols = b*16 + ic*4 + c
            Q_all = pp.tile([128, 64], f32, tag="Q")
            w1_all = pp.tile([128, 64], f32, tag="w1")
            w0_all = pp.tile([128, 16], f32, tag="w0")  # cols = b*4 + ic
            w0T = pp.tile([16, 128], f32, tag="w0T")
            w0f = pp.tile([1, 16 * 128], f32, tag="w0f")
            bvb = pp.tile([128, DK], bf16, tag="bvb")

            wb = cp.tile([128, 1408], bf16, tag="wb")
            wf = cp.tile([128, 132], f32, tag="wf")
            sb = cp.tile([DK, 4], f32, tag="sb")
            bv_s = cp.tile([1, DK], bf16, tag="bv")
            ones_b = cp.tile([1, 128], bf16, tag="ones_b")
            ones_f = cp.tile([1, 128], f32, tag="ones_f")
            warm_z = cp.tile([128, S], bf16, tag="warm_z")

            def Wq_c(mc):
                return wb[:, mc * DK:(mc + 1) * DK]

            def Wk_c(mc):
                return wb[:, 256 + mc * DK:256 + (mc + 1) * DK]

            def Wv_c(mc):
                return wb[:, 512 + mc * DK:512 + (mc + 1) * DK]

            def WoP_c(nch):
                return wb[:, 768 + nch * 128:768 + (nch + 1) * 128]

            id_s = wb[:, 1280:1408]
            idf_s = wf[:, 0:128]
            bqc_s, bkc_s = sb[:, 0:1], sb[:, 1:2]
            al_s, b4_s = sb[:, 2:3], sb[:, 3:4]

            # ---- DMA: big inputs lead the sync queue ----
            ktiles, qtiles, vtiles = [], [None] * BS, [None] * BS
            for b in range(BS):
                kt = kip.tile([128, 4, S], bf16, tag=f"kt{b}")
                nc.sync.dma_start(kt[:], kT_d[b])
                ktiles.append(kt)
            for b in range(BS):
                qt = qip.tile([128, 4, S], bf16, tag="qt", name=f"qt{b}")
                nc.sync.dma_start(qt[:], qT_d[b])
                qtiles[b] = qt
                vt = vip.tile([128, 4, S], bf16, tag=f"vt{b}")
                nc.sync.dma_start(vt[:], vT_d[b])
                vtiles[b] = vt
            # packed weights on the scalar (ACT) HWDGE queue in parallel
            nc.scalar.dma_start(wb[:], wb_d[:])
            nc.scalar.dma_start(wf[:], wf_d[:])
            nc.scalar.dma_start(sb[:], sb_d[:])
            nc.scalar.dma_start(bv_s[:], bv_d[:])

            nc.vector.memset(ones_b[:], 1.0)
            nc.vector.memset(ones_f[:], 1.0)
            nc.vector.memset(warm_z[:], 0.0)

            # ---- single PSUM rotation: [128, 4, 512] x 2 bufs = 8 banks ----
            def psum_t(name="ppe"):
                return psp.tile([128, 4, S], f32, tag="pe", bufs=2, name=name)

            # ---- P1 projections (slot 0 own tile, or slot 3 borrowed) ----
            def emit_proj(W_c, b_c, srct, dst_ap, on_act, ph=None):
                slot = 3 if ph is not None else 0
                ps = ph if ph is not None else psum_t("pproj")
                for mc in range(4):
                    nc.tensor.matmul(ps[0:DK, slot, :], W_c(mc),
                                     srct[:, mc, :],
                                     start=(mc == 0), stop=(mc == 3))
                if on_act:
                    nc.scalar.activation(dst_ap, ps[0:DK, slot, :],
                                         AF.Identity, bias=b_c)
                else:
                    nc.vector.tensor_scalar(dst_ap, ps[0:DK, slot, :], b_c,
                                            None, op0=OP.add)

            def emit_qproj(b, ph=None):
                emit_proj(Wq_c, bqc_s, qtiles[b], qhT[:, b, :], b == 0, ph)
                (nc.scalar if b == 0 else nc.sync).dma_start(
                    hq[64:128, b, :], qhT[:, b, :])

            # ---- bvb: broadcast bv across partitions via K=1 matmul ----
            def emit_bvb():
                pb = psum_t("pbv")
                nc.tensor.matmul(pb[:, 0, 0:DK], ones_b[:, :], bv_s[:],
                                 start=True, stop=True)
                nc.vector.tensor_copy(bvb[:], pb[:, 0, 0:DK])

            # ---- vh per c (borrows slot 1 of a consumed P2 tile) ----
            def emit_vh(c, ph):
                vt = vtiles[c]
                for jc in range(4):
                    for mc in range(4):
                        nc.tensor.matmul(
                            ph[:, 1, jc * DK:(jc + 1) * DK],
                            vt[:, mc, jc * 128:(jc + 1) * 128],
                            Wv_c(mc),
                            start=(mc == 0), stop=(mc == 3),
                        )
                nc.vector.tensor_tensor(
                    vh_all[:, :, c, :],
                    ph[:, 1, 0:4 * DK].rearrange("p (jc d) -> p jc d", d=DK),
                    bvb[:].unsqueeze(1).broadcast_to((128, 4, DK)),
                    op=OP.add)

            # ---- P2 unit (b, jc), software-pipelined halves ----
            def emit_e_part(b, jc):
                ph = psum_t("ppe")
                js = slice(jc * 128, (jc + 1) * 128)
                for kb in range(4):
                    nc.tensor.matmul(ph[:, kb, :], khT[:, kb, js],
                                     qhT[:, b, :], start=True, stop=True)
                ex = exp_pool.tile([128, 4, S], bf16, tag="ex")
                nc.scalar.activation(ex[:], ph[:], AF.Exp)
                return ph, ex

            def emit_fen_part(b, jc, ph, ex):
                for kb in range(4):
                    nc.tensor.matmul(ph[:, 0, :], id_s, ex[:, kb, :],
                                     start=(kb == 0), stop=(kb == 3))
                wr = wrp.tile([128, S], f32, tag="wr")
                nc.vector.reciprocal_approx_fast(wr[:], ph[:, 0, :])
                nc.gpsimd.tensor_tensor(rt_all[:, b, jc, :], ex[:, b, :],
                                        wr[:], op=OP.mult)

            # ---- P3 per b: scores + exp (own tile, post-loop) ----
            def emit_p3_mms_act(b):
                ps3 = psum_t("p3")
                for ic in range(4):
                    for jc in range(4):
                        nc.tensor.matmul(
                            ps3[:, ic // 2, (ic % 2) * 256:(ic % 2 + 1) * 256],
                            rt_all[:, b, jc, ic * 128:(ic + 1) * 128],
                            vh_all[:, jc].rearrange("p c d -> p (c d)"),
                            start=(jc == 0), stop=(jc == 3),
                        )
                nc.scalar.activation(
                    e_all[:, b, 0, :].rearrange("p (a i) -> p a i", a=2),
                    ps3[:, 0:2, :], AF.Exp)

            def emit_p3_dve(b):
                e2eng = nc.gpsimd if b < 2 else nc.vector
                e2eng.tensor_tensor(e_all[:, b, 1, :], e_all[:, b, 0, :],
                                    e_all[:, b, 0, :], op=OP.mult)
                nc.vector.tensor_reduce(
                    Z_all[:, b * 16:(b + 1) * 16],
                    e_all[:, b, 0, :].rearrange("p (g d) -> p g d", d=DK),
                    axis=AX.X, op=OP.add)
                nc.vector.tensor_reduce(
                    Q_all[:, b * 16:(b + 1) * 16],
                    e_all[:, b, 1, :].rearrange("p (g d) -> p g d", d=DK),
                    axis=AX.X, op=OP.add)

            # ---- P4: ONE merged stats chain over all 64 cols ----
            def emit_stats():
                Zs, Qs = Z_all[:, :], Q_all[:, :]
                t = stp.tile([128, 64], f32, tag="t", name="t")
                nc.vector.tensor_tensor(t[:], Zs, Zs, op=OP.mult)
                s = stp.tile([128, 64], f32, tag="s", name="s")
                nc.vector.scalar_tensor_tensor(
                    s[:], t[:], -1.0 / DK, Qs, op0=OP.mult, op1=OP.add)
                rinv = stp.tile([128, 64], f32, tag="rinv", name="rinv")
                nc.vector.reciprocal(rinv[:], t[:])
                v63 = stp.tile([128, 64], f32, tag="v63", name="v63")
                nc.vector.tensor_tensor(v63[:], s[:], rinv[:], op=OP.mult)
                r_ = stp.tile([128, 64], f32, tag="r_", name="r_")
                nc.vector.tensor_scalar(r_[:].bitcast(i32), v63[:].bitcast(i32),
                                        1, None, op0=OP.logical_shift_right)
                nc.vector.tensor_scalar(r_[:].bitcast(i32), r_[:].bitcast(i32),
                                        -1, 0x5F3759DF, op0=OP.mult, op1=OP.add)
                nt = stp.tile([128, 64], f32, tag="nt", name="nt")
                for _ in range(1):
                    nc.vector.tensor_tensor(nt[:], v63[:], r_[:], op=OP.mult)
                    nc.vector.tensor_tensor(nt[:], nt[:], r_[:], op=OP.mult)
                    nc.vector.tensor_scalar(nt[:], nt[:], -0.5, 1.5,
                                            op0=OP.mult, op1=OP.add)
                    nc.vector.tensor_tensor(r_[:], r_[:], nt[:], op=OP.mult)
                R_ = stp.tile([128, 64], f32, tag="R_", name="R_")
                nc.vector.tensor_scalar(R_[:], r_[:], float(np.sqrt(DK - 1.0)),
                                        None, op0=OP.mult)
                u_ = stp.tile([128, 64], f32, tag="u_", name="u_")
                nc.vector.tensor_scalar(u_[:], R_[:], -EPS, 1.0,
                                        op0=OP.mult, op1=OP.add)
                g = stp.tile([128, 64], f32, tag="g", name="g")
                nc.vector.tensor_tensor(g[:], R_[:], u_[:], op=OP.mult)
                zr = stp.tile([128, 64], f32, tag="zr", name="zr")
                nc.vector.reciprocal(zr[:], Zs)
                nc.vector.tensor_tensor(w1_all[:, :], g[:], zr[:], op=OP.mult)
                gs = stp.tile([128, 16], f32, tag="gs", name="gs")
                nc.vector.tensor_reduce(
                    gs[:], g[:].rearrange("p (s c) -> p s c", c=4), axis=AX.X,
                    op=OP.add)
                nc.vector.tensor_scalar(w0_all[:, :], gs[:],
                                        -1.0 / DK, None, op0=OP.mult)
                pw = psum_t("pw")
                nc.tensor.matmul(pw[:16, 0, 0:128], w0_all[:, :],
                                 idf_s, is_transpose=True, start=True,
                                 stop=True)
                nc.vector.tensor_copy(w0T[:, :], pw[:16, 0, 0:128])
                nc.sync.dma_start(
                    w0f[0:1, :].rearrange("o (s f) -> o s f", s=16),
                    w0T[:, :])

            # ---- P5 per b ----
            def emit_p5_prep(b):
                w1e = p5p.tile([128, 16, DK], bf16, tag="w1e")
                nc.vector.tensor_copy(
                    w1e[:],
                    w1_all[:, b * 16:(b + 1) * 16].unsqueeze(-1)
                    .broadcast_to((128, 16, DK)))
                bsc = p5p.tile([128, 4, 4, DK], bf16, tag="bsc")
                nc.vector.tensor_tensor(
                    bsc[:].rearrange("p i c d -> p (i c d)"),
                    e_all[:, b, 0, :],
                    w1e[:].rearrange("p (i c) d -> p (i c d)", c=4),
                    op=OP.mult)
                t01 = p5p.tile([128, 4, DK], bf16, tag="t01")
                nc.gpsimd.tensor_tensor(t01[:], bsc[:, :, 0, :],
                                        bsc[:, :, 1, :], op=OP.add)
                t23 = p5p.tile([128, 4, DK], bf16, tag="t23")
                nc.gpsimd.tensor_tensor(t23[:], bsc[:, :, 2, :],
                                        bsc[:, :, 3, :], op=OP.add)
                ball = p5p.tile([128, 4, DK], bf16, tag="ball")
                nc.gpsimd.tensor_tensor(ball[:], t01[:], t23[:], op=OP.add)
                return ball

            def emit_p5(b, ball):
                pbig = psum_t("p5")
                for ic in range(4):
                    nc.tensor.matmul(pbig[0:64, 0, ic * 128:(ic + 1) * 128],
                                     ball[:, ic, :], id_s,
                                     start=True, stop=False,
                                     skip_group_check=True)
                    slot = b * 4 + ic
                    nc.tensor.matmul(
                        pbig[0:64, 0, ic * 128:(ic + 1) * 128],
                        ones_f[:, 0:DK],
                        w0f[0:1, slot * 128:(slot + 1) * 128],
                        start=False, stop=True, skip_group_check=True,
                    )
                nc.vector.tensor_scalar(
                    hq[0:64, b, :], pbig[0:64, 0, :],
                    al_s, b4_s, op0=OP.mult, op1=OP.add,
                )

            # ---- P6 per b: out = WoP^T @ [heads; qhT] (+bo), copies on ACT
            def emit_p6(b):
                po = psum_t("p6")
                for nch in range(4):
                    nc.tensor.matmul(po[:, nch, :], WoP_c(nch),
                                     hq[:, b, :], start=True, stop=True)
                ot = otp.tile([128, 4, S], bf16, tag="ot")
                if bo_zero:
                    nc.scalar.activation(ot[:], po[:], AF.Identity)
                else:
                    for nch in range(4):
                        nc.scalar.activation(ot[:, nch, :], po[:, nch, :],
                                             AF.Identity,
                                             bias=wf[:, 128 + nch:129 + nch])
                nc.sync.dma_start(
                    outT_d[b].rearrange("(n p) i -> p n i", p=128), ot[:])

            # ---- emission schedule ----
            for kb in range(4):
                emit_proj(Wk_c, bkc_s, ktiles[kb], khT[:, kb, :], True)
                if kb == 1:
                    # Exp table load in the ACT gap between k-proj copies
                    nc.scalar.activation(warm_z[0:1, 0:8], warm_z[0:1, 0:8],
                                         AF.Exp)
            emit_qproj(0)
            emit_bvb()

            # extras at loop index i borrow the (just consumed) PSUM tile of
            # unit i-1: vh uses slot 1, q-proj uses slot 3 — preserving the
            # 2-buffer rotation parity of the E-unit stream.
            extras = {
                3: [lambda ph: emit_qproj(1, ph)],
                7: [lambda ph: emit_qproj(2, ph)],
                8: [lambda ph: emit_vh(0, ph)],
                10: [lambda ph: emit_qproj(3, ph),
                     lambda ph: emit_vh(1, ph)],
                12: [lambda ph: emit_vh(2, ph)],
                14: [lambda ph: emit_vh(3, ph)],
            }
            units = [(b, jc) for b in range(BS) for jc in range(4)]
            pend = None
            for i, (b, jc) in enumerate(units):
                ph, ex = emit_e_part(b, jc)
                if pend is not None:
                    emit_fen_part(*pend)
                    for fn in extras.get(i, []):
                        fn(pend[2])
                pend = (b, jc, ph, ex)
            emit_fen_part(*pend)

            # ---- tail: P3 pipeline, stats, prepped P5/P6 pipeline ----
            emit_p3_mms_act(0)
            emit_p3_mms_act(1)
            emit_p3_dve(0)
            emit_p3_mms_act(2)
            emit_p3_dve(1)
            emit_p3_mms_act(3)
            emit_p3_dve(2)
            emit_p3_dve(3)
            emit_stats()
            balls = [emit_p5_prep(b) for b in range(BS)]
            for b in range(BS):
                emit_p5(b, balls[b])
                emit_p6(b)

    return nc


def _build(bo_zero):
    import concourse.bass as bass  # noqa
    import concourse.tile as tile
    from concourse import bacc, mybir

    nc = bacc.Bacc("TRN2", target_bir_lowering=False, debug=False,
                   num_devices=NCORES)
    build_program(nc, tile, mybir, bo_zero)
    nc.compile()
    return nc


_cached_nc = None
_cached_bo_zero = None


def make_in_maps(q, k, v, Wq, bq, Wk, bk, Wv, bv, Wo, bo, alpha, beta):
    import ml_dtypes
    bft = ml_dtypes.bfloat16

    def prelay(x):
        # [S, DM] per batch -> transposed [DM, S] -> [128, 4, S] layout
        xT = np.swapaxes(np.asarray(x, np.float32), 1, 2)  # [B, DM, S]
        return np.ascontiguousarray(
            xT.reshape(BS, 4, 128, S).transpose(0, 2, 1, 3)).astype(bft)

    def wlay(W):  # [DM, DK] -> [128, (4, DK)]
        return np.ascontiguousarray(
            np.asarray(W, np.float32).reshape(4, 128, DK).transpose(1, 0, 2)
        ).reshape(128, 4 * DK)

    qT, kT, vT = prelay(q), prelay(k), prelay(v)
    Wq, Wk, Wv, Wo = (np.asarray(x, np.float32) for x in (Wq, Wk, Wv, Wo))
    bq, bk, bv, bo = (np.asarray(x, np.float32) for x in (bq, bk, bv, bo))
    alpha, beta = np.asarray(alpha, np.float32), np.asarray(beta, np.float32)
    scale = np.float32(1.0 / np.sqrt(np.float32(DK)))  # fenmu sqrt(DK) -> Wv
    in_maps = []
    for h in range(NCORES):
        sl = slice(h * DK, (h + 1) * DK)
        wbp = np.zeros((128, 1408), np.float32)
        wbp[:, 0:256] = wlay(Wq[:, sl])
        wbp[:, 256:512] = wlay(Wk[:, sl])
        wbp[:, 512:768] = wlay(Wv[:, sl] * scale)
        for nch in range(4):
            wbp[0:64, 768 + nch * 128:768 + (nch + 1) * 128] = \
                Wo[sl, nch * 128:(nch + 1) * 128]
            wbp[64:128, 768 + nch * 128:768 + (nch + 1) * 128] = \
                4.0 * Wo[sl, nch * 128:(nch + 1) * 128]
        wbp[:, 1280:1408] = np.eye(128, dtype=np.float32)
        wfp = np.zeros((128, 132), np.float32)
        wfp[:, 0:128] = np.eye(128, dtype=np.float32)
        wfp[:, 128:132] = (bo if h == 0 else np.zeros_like(bo)).reshape(
            4, 128).T
        sbp = np.stack([bq[sl], bk[sl], alpha, 4.0 * beta], axis=1)
        in_maps.append({
            "qT": qT, "kT": kT, "vT": vT,
            "wb": wbp.astype(bft),
            "wf": wfp.astype(np.float32),
            "sb": np.ascontiguousarray(sbp).astype(np.float32),
            "bv": np.ascontiguousarray(bv[sl] * scale)[None, :].astype(bft),
        })
    return in_maps


def assemble(results):
    out = np.zeros((BS, S, DM), np.float32)
    for r in results:
        out += np.swapaxes(np.asarray(r["outT"], np.float32), 1, 2)
    return out


def kernel(**inputs) -> np.ndarray:
    global _cached_nc, _cached_bo_zero
    from concourse.bass_utils import run_bass_kernel_spmd

    bo_zero = bool(np.all(np.asarray(inputs["bo"]) == 0.0))
    if _cached_nc is None or _cached_bo_zero != bo_zero:
        _cached_nc = _build(bo_zero)
        _cached_bo_zero = bo_zero
    in_maps = make_in_maps(**inputs)
    res = run_bass_kernel_spmd(_cached_nc, in_maps, list(range(NCORES)))
    return assemble(res.results)
